# revision 24
# baseline (speedup 1.0000x reference)
"""GQA attention (dense_transformer) TRN2 Bass kernel — 8 NeuronCores.

Problem: b=2, s=2048, d=2048, nh=16, nkv=4, hd=128, causal GQA attention
block with RMS-normed+RoPE'd q/k and per-head q gains.

Sharding: batch DP=2 x head TP=4  ->  8 cores. Each core handles one batch
element, 4 q heads, 1 kv head. Wq/Wk/Wv column-sharded, Wo row-sharded;
partial outputs summed on host.

v2 dataflow per core (matmuls bf16 in / fp32 PSUM):
  1. Projections streamed in 512-wide s-column blocks (xT arrives in 4
     DMA waves; rotating SBUF block buffer). Per 128-row s-tile: Q then
     KV matmul chains over d; RMS stats via two batched Squares (ScalarE)
     + one grouped free-dim reduce (DVE); bit-trick rsqrt; rope applied
     straight out of PSUM via scalar_tensor_tensor; PE-transpose to
     [hd, s].
  2. Attention per (q-chunk, head): scores S^T = K-tile @ Q-chunk in
     kt-PAIRS into a 2-bank PSUM tile so one Exp covers 1024 columns;
     causal diag tiles masked on DVE; A@V computed per 128-q-subtile
     with pt as the stationary operand and [V | 1] as the 129-wide
     moving operand, so the softmax denominator rides along as PSUM
     column 128 (no separate row-sum matmul). Normalize = per-partition
     reciprocal + tensor_scalar; PE-transpose back to [hd, s].
  3. Output projection interleaved between attention heads of the next
     q-chunk (keeps ScalarE exp backlog off the critical path and
     spreads outT DMAs).
"""

import math
import sys

if "/opt/trn_rl_repo" not in sys.path:
    sys.path.insert(0, "/opt/trn_rl_repo")

import numpy as np
import ml_dtypes

import concourse.mybir as mybir
import concourse.tile as tile
from concourse.bass_types import AP
from concourse import bacc
from concourse.bass_utils import run_bass_kernel_spmd

F32 = mybir.dt.float32
I32 = mybir.dt.int32
BF16 = mybir.dt.bfloat16
AF = mybir.ActivationFunctionType
ALU = mybir.AluOpType
AXL = mybir.AxisListType

BF16NP = ml_dtypes.bfloat16
RMS_EPS = float(np.finfo(np.float32).eps)

S, D, NQ, HD = 2048, 2048, 4, 128
DQ = NQ * HD            # 512: per-core q width
NTP = 4                 # tensor-parallel ways (heads)
NB = 2                  # batch (data-parallel ways)
NCORES = 8

_NC_CACHE = {}


def build_kernel(S=S, D=D, NQ=NQ, HD=HD, num_devices=NCORES):
    DQ = NQ * HD
    NST = S // 128          # s-tiles
    NDC = D // 128          # d-chunks (projection contraction)
    NQC = S // 512          # q-chunks for attention
    NBLK = 4                # xT streaming blocks (512 s-cols each)
    scale = 1.0 / math.sqrt(HD)
    H = HD // 2
    NH1 = NQ + 1
    # offsets of the four [128,129] AV accumulators inside the 2-bank pyp
    # tile (16B aligned, none crossing a PSUM bank boundary); transposed
    # output parks at [644:900) as bf16.
    PYOFF = (0, 144, 288, 512)

    nc = bacc.Bacc("TRN2", target_bir_lowering=False, debug=False,
                   num_devices=num_devices)

    xT = nc.dram_tensor("xT", [D, S], BF16, kind="ExternalInput").ap()
    wq = nc.dram_tensor("wq", [D, DQ], BF16, kind="ExternalInput").ap()
    wkv = nc.dram_tensor("wkv", [D, 2 * HD], BF16, kind="ExternalInput").ap()
    wo = nc.dram_tensor("wo", [DQ, D], BF16, kind="ExternalInput").ap()
    cst = nc.dram_tensor("cst", [S, 2 * HD], BF16, kind="ExternalInput").ap()
    qgb = nc.dram_tensor("qgb", [128, 4 * (NQ + 1)], F32,
                         kind="ExternalInput").ap()
    ident = nc.dram_tensor("ident", [128, 128], BF16, kind="ExternalInput").ap()
    msk = nc.dram_tensor("msk", [128, 4 * 512], BF16, kind="ExternalInput").ap()
    ones = nc.dram_tensor("ones", [128, 128], BF16, kind="ExternalInput").ap()
    outT = nc.dram_tensor("outT", [D, S], F32, kind="ExternalOutput").ap()

    with tile.TileContext(nc) as tc:
        from contextlib import ExitStack
        with ExitStack() as ctx:
            consts = ctx.enter_context(tc.tile_pool(name="consts", bufs=1))
            wpool = ctx.enter_context(tc.tile_pool(name="w", bufs=1))
            xpool = ctx.enter_context(tc.tile_pool(name="xT", bufs=2))
            qt_pool = ctx.enter_context(tc.tile_pool(name="qt", bufs=1))
            yt_pool = ctx.enter_context(tc.tile_pool(name="yt", bufs=1))
            v_pool = ctx.enter_context(tc.tile_pool(name="vrow", bufs=1))
            sq_pool = ctx.enter_context(tc.tile_pool(name="sq", bufs=3))
            st_pool = ctx.enter_context(tc.tile_pool(name="stat", bufs=3))
            tv_pool = ctx.enter_context(tc.tile_pool(name="tv", bufs=2))
            ro_pool = ctx.enter_context(tc.tile_pool(name="ro", bufs=2))
            pt_pool = ctx.enter_context(tc.tile_pool(name="ptile", bufs=6))
            rn_pool = ctx.enter_context(tc.tile_pool(name="rn", bufs=3))
            ob_pool = ctx.enter_context(tc.tile_pool(name="ob", bufs=4))
            # PSUM: pA 2x[128,1024] (4 banks) + pB 1x[128,1024] (2 banks)
            # + pC 2x[128,512] (2 banks) = all 8 banks.
            pA = ctx.enter_context(tc.tile_pool(name="pA", bufs=2, space="PSUM"))
            pB = ctx.enter_context(tc.tile_pool(name="pB", bufs=1, space="PSUM"))
            pC = ctx.enter_context(tc.tile_pool(name="pC", bufs=2, space="PSUM"))

            # ---- weight/const DMAs interleaved with the first xT block so
            # the first Q matmuls start ~2us in.
            xTr = xT.rearrange("(n p) m -> p n m", p=128)
            wqr = wq.rearrange("(n p) m -> p n m", p=128)
            wkvr = wkv.rearrange("(n p) m -> p n m", p=128)
            wq_sb = wpool.tile([128, NDC, DQ], BF16, tag="wq")
            wkv_sb = wpool.tile([128, NDC, 2 * HD], BF16, tag="wkv")

            x_blocks = [None] * NBLK
            x_blocks[0] = xpool.tile([128, NDC, 512], BF16, name="xb0",
                                     tag="xb")
            ident_sb = consts.tile([128, 128], BF16, tag="ident")
            nc.sync.dma_start(ident_sb[:], ident)
            nc.sync.dma_start(wq_sb[:, 0:2, :], wqr[:, 0:2, :])
            nc.sync.dma_start(x_blocks[0][:, 0:2, :], xTr[:, 0:2, 0:512])
            nc.sync.dma_start(wq_sb[:, 2:4, :], wqr[:, 2:4, :])
            nc.sync.dma_start(x_blocks[0][:, 2:4, :], xTr[:, 2:4, 0:512])
            nc.sync.dma_start(wq_sb[:, 4:8, :], wqr[:, 4:8, :])
            nc.sync.dma_start(x_blocks[0][:, 4:8, :], xTr[:, 4:8, 0:512])
            nc.sync.dma_start(wq_sb[:, 8:NDC, :], wqr[:, 8:NDC, :])
            nc.sync.dma_start(x_blocks[0][:, 8:NDC, :], xTr[:, 8:NDC, 0:512])
            nc.sync.dma_start(wkv_sb[:], wkvr)

            cst_sb = consts.tile([128, NST, 2 * HD], BF16, tag="cst")
            nc.sync.dma_start(cst_sb[:],
                              cst.rearrange("(n p) m -> p n m", p=128))
            qgb_sb = consts.tile([128, 4 * (NQ + 1)], F32, tag="qgb")
            nc.sync.dma_start(qgb_sb[:], qgb)
            ones_sb = consts.tile([128, 128], BF16, tag="ones")
            nc.sync.dma_start(ones_sb[:], ones)
            msk_sb = consts.tile([128, 4, 512], BF16, tag="msk")
            nc.sync.dma_start(msk_sb[:], msk.rearrange("p (m c) -> p m c", c=512))
            wo_sb = wpool.tile([128, NQ, D], BF16, tag="wo")
            nc.sync.dma_start(wo_sb[:], wo.rearrange("(n p) m -> p n m", p=128))

            # HAM warmup: ~4.5us of PE work with NO input dependency (reads
            # an uninitialized SBUF scratch tile) so the clock gate is at 8/8
            # and the PE pipeline primed when the first real matmuls arrive.
            wsrc = consts.tile([128, 128], BF16, tag="wsrc")
            nc.vector.memset(wsrc[:], 1.0)
            warm = pC.tile([128, 512], F32, name="warm", tag="c")
            for i in range(60):
                nc.tensor.matmul(warm[:, 0:128], wsrc[:], wsrc[:],
                                 start=True, stop=True)

            qt_all = qt_pool.tile([128, NH1, S], BF16, name="qt_all",
                                  tag="qt_all")
            yt_tiles = [yt_pool.tile([128, S], BF16, name=f"yt{h}", tag=f"yt{h}")
                        for h in range(NQ)]
            v_tiles = [v_pool.tile([128, 132], BF16, name=f"v{st}", tag=f"v{st}")
                       for st in range(NST)]

            # ---- Phase 1: projections + rms-norm + rope + transpose ----
            # The PE transposes of s-tile st are deferred until the next
            # s-tile's projection matmuls have been emitted, so the DVE rope
            # chain has a full tile of slack before the PE needs its output.
            tr_state = {"pend": None}

            def flush_tr():
                if tr_state["pend"] is None:
                    return
                st, ro5 = tr_state["pend"]
                tr_state["pend"] = None
                bt = pB.tile([128, 1024], F32, name="bt", tag="b")
                ptv = bt[:, 0:NH1 * 64].bitcast(BF16)  # [128, 640] bf16
                for i in range(NH1):
                    nc.tensor.transpose(ptv[:, i * 128:(i + 1) * 128],
                                        ro5[:, i * HD:(i + 1) * HD],
                                        ident_sb[:])
                nc.scalar.copy(
                    qt_all[:, :, st * 128:(st + 1) * 128],
                    ptv.rearrange("p (h c) -> p h c", c=128))

            def process_stile(st, xb, st4):
                at = pA.tile([128, 1024], F32, name="at", tag="a")
                pq = at[:, 0:DQ]
                pkv = at[:, DQ:DQ + 2 * HD]
                for dc in range(NDC):
                    nc.tensor.matmul(pq, xb[:, dc, st4 * 128:(st4 + 1) * 128],
                                     wq_sb[:, dc, :],
                                     start=dc == 0, stop=dc == NDC - 1)
                for dc in range(NDC):
                    nc.tensor.matmul(pkv, xb[:, dc, st4 * 128:(st4 + 1) * 128],
                                     wkv_sb[:, dc, :],
                                     start=dc == 0, stop=dc == NDC - 1)
                flush_tr()

                # evacuate PSUM on ScalarE (frees the accumulator banks
                # fast; the slow DVE chain then runs from SBUF)
                qkv = sq_pool.tile([128, DQ + 2 * HD], BF16, tag="qkv")
                nc.scalar.copy(qkv[:, 0:DQ], pq)
                nc.vector.tensor_copy(qkv[:, DQ:DQ + 2 * HD], pkv)

                # V row tile [v | 1] for the AV' matmuls
                nc.vector.tensor_copy(v_tiles[st][:, 0:HD],
                                      qkv[:, DQ + HD:DQ + 2 * HD])
                nc.vector.tensor_copy(v_tiles[st][:, HD:HD + 1],
                                      ones_sb[:, 0:1])

                # RMS stats: one batched square + one grouped reduce
                sq = sq_pool.tile([128, NH1 * HD], F32, tag="sq")
                nc.scalar.activation(sq[:], qkv[:, 0:DQ + HD], AF.Square)
                ssq = st_pool.tile([128, NH1], F32, tag="ssq")
                sq_g = AP(sq.tensor, sq.offset, [sq.ap[0], [HD, NH1], [1, HD]])
                nc.vector.tensor_reduce(ssq[:], sq_g, axis=AXL.X, op=ALU.add)

                # rinv = (mean(q^2)+eps)**-0.5 on DVE (bit-trick + 2 Newton)
                m = st_pool.tile([128, NH1], F32, tag="m")
                nc.vector.tensor_scalar(m[:], ssq[:], 1.0 / HD, RMS_EPS,
                                        op0=ALU.mult, op1=ALU.add)
                y0 = st_pool.tile([128, NH1], F32, tag="y0")
                nc.vector.tensor_scalar(y0[:].bitcast(I32),
                                        m[:].bitcast(I32), 1, None,
                                        op0=ALU.arith_shift_right)
                nc.vector.tensor_scalar(y0[:].bitcast(I32),
                                        y0[:].bitcast(I32),
                                        -1, 0x5F3759DF,
                                        op0=ALU.mult, op1=ALU.add)
                rinv = y0
                for _ in range(1):
                    aa = st_pool.tile([128, NH1], F32, tag="nr_a")
                    nc.vector.tensor_mul(aa[:], rinv[:], rinv[:])
                    nc.vector.tensor_mul(aa[:], aa[:], m[:])
                    nc.vector.tensor_scalar(aa[:], aa[:], -0.5, 1.5,
                                            op0=ALU.mult, op1=ALU.add)
                    nxt = st_pool.tile([128, NH1], F32, tag="nr_y")
                    nc.vector.tensor_mul(nxt[:], rinv[:], aa[:])
                    rinv = nxt
                nc.vector.tensor_mul(rinv[:], rinv[:], qgb_sb[:, 0:NH1])

                # rope straight from PSUM: per head [t|v] = (q*rinv)*[c|c|-s|s]
                cst_t = cst_sb[:, st, :]
                tv5 = tv_pool.tile([128, NH1 * 2 * HD], BF16, tag="tv5")
                for i in range(NH1):
                    q_ap = qkv[:, i * HD:(i + 1) * HD]
                    q_rep = AP(q_ap.tensor, q_ap.offset,
                               [q_ap.ap[0], [0, 2], [1, HD]])
                    nc.vector.scalar_tensor_tensor(
                        tv5[:, i * 2 * HD:(i + 1) * 2 * HD],
                        q_rep, rinv[:, i:i + 1], cst_t,
                        op0=ALU.mult, op1=ALU.mult)
                ro5 = ro_pool.tile([128, NH1 * HD], BF16, tag="ro5")
                b5 = tv5[:]
                t_view = AP(b5.tensor, b5.offset,
                            [b5.ap[0], [2 * HD, NH1], [H, 2], [1, H]])
                v_view = AP(b5.tensor, b5.offset + HD + H,
                            [b5.ap[0], [2 * HD, NH1], [-H, 2], [1, H]])
                r5 = ro5[:]
                o_view = AP(r5.tensor, r5.offset,
                            [r5.ap[0], [HD, NH1], [H, 2], [1, H]])
                nc.vector.tensor_add(o_view, t_view, v_view)
                tr_state["pend"] = (st, ro5)

            # ---- Phases interleaved: projections block b -> attention
            # qc=b -> outproj qc=b-1 as PE filler between heads ----
            kt_row = qt_all[:, NQ, :]

            # out-projection dribbled one 128x512 tile at a time between
            # attention score groups (PE filler while ScalarE exps run);
            # outT DMA batched per 4 tiles.
            op_state = {"pending": [], "ob": None, "row": 0}

            def push_outproj(qcp):
                op_state["pending"].extend((qcp, dt) for dt in range(NST))

            def emit_outproj_unit(copy_eng="vector"):
                if not op_state["pending"]:
                    return
                qcp, dt = op_state["pending"].pop(0)
                if op_state["row"] == 0:
                    op_state["ob"] = ob_pool.tile([128, 4, 512], F32,
                                                  name="ob", tag="ob")
                po = pC.tile([128, 512], F32, name="po", tag="c")
                for dqc in range(NQ):
                    nc.tensor.matmul(
                        po[:], wo_sb[:, dqc, dt * 128:(dt + 1) * 128],
                        yt_tiles[dqc][:, qcp * 512:(qcp + 1) * 512],
                        start=(dqc == 0), stop=(dqc == NQ - 1))
                if copy_eng == "scalar":
                    nc.scalar.copy(op_state["ob"][:, op_state["row"], :], po[:])
                else:
                    nc.vector.tensor_copy(
                        op_state["ob"][:, op_state["row"], :], po[:])
                op_state["row"] += 1
                if op_state["row"] == 4:
                    op_state["row"] = 0
                    nc.sync.dma_start(
                        outT[(dt - 3) * 128:(dt + 1) * 128,
                             qcp * 512:(qcp + 1) * 512].rearrange(
                                 "(n p) m -> p n m", p=128),
                        op_state["ob"][:])

            work_q = []  # pending s-tile closures (consumed as PE filler)

            def fill_slot():
                if work_q:
                    work_q.pop(0)()
                else:
                    emit_outproj_unit()

            def attention_head(qc, h):
                n_kt = 4 * qc + 4
                n_groups = n_kt // 2
                qs = qt_all[:, h, qc * 512:(qc + 1) * 512]

                def off_of(kt):
                    return max(0, kt - 4 * qc) * 128

                def emit_scores_group(g):
                    sp = pA.tile([128, 1024], F32, name="sp", tag="a")
                    ptp = pt_pool.tile([128, 2, 512], BF16, name="ptp",
                                       tag="ptp")
                    for u in (0, 1):
                        kt = 2 * g + u
                        off = off_of(kt)
                        nc.tensor.matmul(
                            sp[:, u * 512 + off:(u + 1) * 512],
                            kt_row[:, kt * 128:(kt + 1) * 128],
                            qs[:, off:512], start=True, stop=True)
                    if 2 * g + 1 < 4 * qc:  # both tiles non-diagonal
                        nc.scalar.activation(
                            ptp[:].rearrange("p a b -> p (a b)"),
                            sp[:], AF.Exp, scale=scale)
                    else:
                        for u in (0, 1):
                            kt = 2 * g + u
                            off = off_of(kt)
                            nc.scalar.activation(ptp[:, u, off:512],
                                                 sp[:, u * 512 + off:(u + 1) * 512],
                                                 AF.Exp, scale=scale)
                    for u in (0, 1):
                        kt = 2 * g + u
                        mdiag = kt - 4 * qc
                        if mdiag >= 0:
                            off = mdiag * 128
                            nc.vector.tensor_mul(
                                ptp[:, u, off:512], ptp[:, u, off:512],
                                msk_sb[:, mdiag, off:512])
                    return ptp

                def emit_av_group(g, ptp, pyp):
                    for u in (0, 1):
                        kt = 2 * g + u
                        j0 = max(0, kt - 4 * qc)
                        for j in range(j0, 4):
                            nc.tensor.matmul(
                                pyp[:, PYOFF[j]:PYOFF[j] + HD + 1],
                                ptp[:, u, j * 128:(j + 1) * 128],
                                v_tiles[kt][:, 0:HD + 1],
                                start=(kt == 0 and j in (0, 3)),
                                stop=(kt == 4 * qc + j),
                                skip_group_check=True)

                prev = emit_scores_group(0)
                fill_slot()
                # start=True on any matmul clears has_written for its whole
                # PSUM bank, which would wipe sibling accumulators sharing
                # the bank -- so zero the regions once and accumulate with
                # start=False throughout.
                pyp = pB.tile([128, 1024], F32, name="pyp", tag="b")
                for g in range(1, n_groups):
                    cur = emit_scores_group(g)
                    emit_av_group(g - 1, prev, pyp)
                    prev = cur
                    if qc < 2 or g % 2 == 1:
                        fill_slot()
                emit_av_group(n_groups - 1, prev, pyp)

                # normalize: rcp of the 4 denominator columns, then per-
                # partition scale of each [q,hd] block; transpose to [hd,q].
                rcp = rn_pool.tile([128, 4], F32, tag="rcp")
                r3 = AP(pyp.tensor, pyp.offset + HD,
                        [pyp.ap[0], [PYOFF[1] - PYOFF[0], 3]])
                nc.vector.reciprocal(out=rcp[:, 0:3], in_=r3)
                nc.vector.reciprocal(out=rcp[:, 3:4],
                                     in_=pyp[:, PYOFF[3] + HD:PYOFF[3] + HD + 1])
                yn = rn_pool.tile([128, NQ * HD], BF16, tag="yn")
                for j in range(4):
                    nc.vector.tensor_scalar(
                        yn[:, j * HD:(j + 1) * HD],
                        pyp[:, PYOFF[j]:PYOFF[j] + HD],
                        rcp[:, j:j + 1], None, op0=ALU.mult)
                ytr = pyp[:, 656:912].bitcast(BF16)  # [128, 512] bf16
                for j in range(4):
                    nc.tensor.transpose(ytr[:, j * 128:(j + 1) * 128],
                                        yn[:, j * HD:(j + 1) * HD],
                                        ident_sb[:])
                nc.vector.tensor_copy(
                    yt_tiles[h][:, qc * 512:(qc + 1) * 512], ytr)

            def prefetch_xb(b):
                x_blocks[b] = xpool.tile([128, NDC, 512], BF16,
                                         name=f"xb{b}", tag="xb")
                nc.sync.dma_start(x_blocks[b][:, 0:8, :],
                                  xTr[:, 0:8, b * 512:(b + 1) * 512])
                nc.sync.dma_start(x_blocks[b][:, 8:NDC, :],
                                  xTr[:, 8:NDC, b * 512:(b + 1) * 512])

            # stage A: s-tiles 0-4 directly (one past block 0, so the
            # 1-deep transpose deferral never leaves a tile the next stage's
            # heads need pending at a stage boundary)
            prefetch_xb(1)
            for st in range(5):
                process_stile(st, x_blocks[st // 4], st % 4)

            # stages B..E: attention for qc in order; projection s-tiles of
            # later blocks and out-projection tiles serve as PE filler inside
            # the heads (supply matched to the heads' fill slots).
            STAGE_STILES = {0: range(5, 12), 1: range(12, 16),
                            2: range(0, 0), 3: range(0, 0)}
            for qc in range(NQC):
                if qc == 0:
                    prefetch_xb(2)
                    prefetch_xb(3)
                for st in STAGE_STILES[qc]:
                    work_q.append(
                        lambda st=st: process_stile(st, x_blocks[st // 4],
                                                    st % 4))
                if qc == NQC - 1:
                    flush_tr()  # qc3's heads need st15's qt
                for h in range(NQ):
                    attention_head(qc, h)
                while work_q:  # any unconsumed s-tiles must land before the
                    work_q.pop(0)()  # next stage's heads read their qt/v
                push_outproj(qc)
            while op_state["pending"]:
                emit_outproj_unit()

    nc.compile()
    return nc


def get_nc():
    if "nc" not in _NC_CACHE:
        _NC_CACHE["nc"] = build_kernel()
    return _NC_CACHE["nc"]


def rope_tables(S=S, HD=HD):
    """Packed rope table [S, 2*HD]: [c | c | -s | s]."""
    f = 1.0 / (10000.0 ** (np.arange(0, HD, 2, dtype=np.float32) / HD))
    fr = np.outer(np.arange(S, dtype=np.float32), f)
    c = np.cos(fr).astype(np.float32)
    s = np.sin(fr).astype(np.float32)
    return np.concatenate([c, c, -s, s], axis=1)


def make_in_maps(x, Wq, Wk, Wv, Wo, qg):
    x = np.asarray(x, np.float32)
    Wq = np.asarray(Wq, np.float32)
    Wk = np.asarray(Wk, np.float32)
    Wv = np.asarray(Wv, np.float32)
    Wo = np.asarray(Wo, np.float32)
    qg = np.asarray(qg, np.float32)
    cst = rope_tables()
    ident = np.eye(128, dtype=BF16NP)
    ones = np.ones((128, 128), dtype=BF16NP)
    pp, ff = np.arange(128)[:, None], np.arange(512)[None, :]
    msk = np.concatenate(
        [(ff >= pp + 128 * mm).astype(BF16NP) for mm in range(4)], axis=1)
    xT = [np.ascontiguousarray(x[b].T).astype(BF16NP) for b in range(NB)]
    in_maps = []
    for b in range(NB):
        for tp in range(NTP):
            qgb_row = np.broadcast_to(
                np.concatenate([qg[tp * NQ:(tp + 1) * NQ],
                                [np.float32(1.0)]] * 4)[None, :].astype(np.float32),
                (128, 4 * (NQ + 1))).copy()
            wkv = np.concatenate([
                Wk[tp * HD:(tp + 1) * HD, :].T,
                Wv[tp * HD:(tp + 1) * HD, :].T], axis=1)
            in_maps.append({
                "xT": xT[b],
                "wq": np.ascontiguousarray(
                    Wq[tp * DQ:(tp + 1) * DQ, :].T).astype(BF16NP),
                "wkv": np.ascontiguousarray(wkv).astype(BF16NP),
                "wo": np.ascontiguousarray(
                    Wo[:, tp * DQ:(tp + 1) * DQ].T).astype(BF16NP),
                "cst": cst.astype(BF16NP),
                "qgb": qgb_row,
                "ident": ident,
                "ones": ones,
                "msk": msk,
            })
    return in_maps


def run(x, Wq, Wk, Wv, Wo, qg, trace=False, **trace_kwargs):
    nc = get_nc()
    in_maps = make_in_maps(x, Wq, Wk, Wv, Wo, qg)
    res = run_bass_kernel_spmd(nc, in_maps, core_ids=list(range(NCORES)),
                               trace=trace, **trace_kwargs)
    out = np.empty((NB, S, D), np.float32)
    for b in range(NB):
        acc = res.results[b * NTP]["outT"].astype(np.float32)
        for tp in range(1, NTP):
            acc = acc + res.results[b * NTP + tp]["outT"]
        out[b] = acc.T
    return out, res


def kernel(x, Wq, Wk, Wv, Wo, qg):
    out, _ = run(x, Wq, Wk, Wv, Wo, qg)
    return out


# revision 25
# speedup vs baseline: 1.0014x; 1.0014x over previous
"""GQA attention (dense_transformer) TRN2 Bass kernel — 8 NeuronCores.

Problem: b=2, s=2048, d=2048, nh=16, nkv=4, hd=128, causal GQA attention
block with RMS-normed+RoPE'd q/k and per-head q gains.

Sharding: batch DP=2 x head TP=4  ->  8 cores. Each core handles one batch
element, 4 q heads, 1 kv head. Wq/Wk/Wv column-sharded, Wo row-sharded;
partial outputs summed on host.

v2 dataflow per core (matmuls bf16 in / fp32 PSUM):
  1. Projections streamed in 512-wide s-column blocks (xT arrives in 4
     DMA waves; rotating SBUF block buffer). Per 128-row s-tile: Q then
     KV matmul chains over d; RMS stats via two batched Squares (ScalarE)
     + one grouped free-dim reduce (DVE); bit-trick rsqrt; rope applied
     straight out of PSUM via scalar_tensor_tensor; PE-transpose to
     [hd, s].
  2. Attention per (q-chunk, head): scores S^T = K-tile @ Q-chunk in
     kt-PAIRS into a 2-bank PSUM tile so one Exp covers 1024 columns;
     causal diag tiles masked on DVE; A@V computed per 128-q-subtile
     with pt as the stationary operand and [V | 1] as the 129-wide
     moving operand, so the softmax denominator rides along as PSUM
     column 128 (no separate row-sum matmul). Normalize = per-partition
     reciprocal + tensor_scalar; PE-transpose back to [hd, s].
  3. Output projection interleaved between attention heads of the next
     q-chunk (keeps ScalarE exp backlog off the critical path and
     spreads outT DMAs).
"""

import math
import sys

if "/opt/trn_rl_repo" not in sys.path:
    sys.path.insert(0, "/opt/trn_rl_repo")

import numpy as np
import ml_dtypes

import concourse.mybir as mybir
import concourse.tile as tile
from concourse.bass_types import AP
from concourse import bacc
from concourse.bass_utils import run_bass_kernel_spmd

F32 = mybir.dt.float32
I32 = mybir.dt.int32
BF16 = mybir.dt.bfloat16
AF = mybir.ActivationFunctionType
ALU = mybir.AluOpType
AXL = mybir.AxisListType

BF16NP = ml_dtypes.bfloat16
RMS_EPS = float(np.finfo(np.float32).eps)

S, D, NQ, HD = 2048, 2048, 4, 128
DQ = NQ * HD            # 512: per-core q width
NTP = 4                 # tensor-parallel ways (heads)
NB = 2                  # batch (data-parallel ways)
NCORES = 8

_NC_CACHE = {}


def build_kernel(S=S, D=D, NQ=NQ, HD=HD, num_devices=NCORES):
    DQ = NQ * HD
    NST = S // 128          # s-tiles
    NDC = D // 128          # d-chunks (projection contraction)
    NQC = S // 512          # q-chunks for attention
    NBLK = 4                # xT streaming blocks (512 s-cols each)
    scale = 1.0 / math.sqrt(HD)
    H = HD // 2
    NH1 = NQ + 1
    # offsets of the four [128,129] AV accumulators inside the 2-bank pyp
    # tile (16B aligned, none crossing a PSUM bank boundary); transposed
    # output parks at [644:900) as bf16.
    PYOFF = (0, 144, 288, 512)

    nc = bacc.Bacc("TRN2", target_bir_lowering=False, debug=False,
                   num_devices=num_devices)

    xT = nc.dram_tensor("xT", [D, S], BF16, kind="ExternalInput").ap()
    wq = nc.dram_tensor("wq", [D, DQ], BF16, kind="ExternalInput").ap()
    wkv = nc.dram_tensor("wkv", [D, 2 * HD], BF16, kind="ExternalInput").ap()
    wo = nc.dram_tensor("wo", [DQ, D], BF16, kind="ExternalInput").ap()
    cst = nc.dram_tensor("cst", [S, 2 * HD], BF16, kind="ExternalInput").ap()
    qgb = nc.dram_tensor("qgb", [128, 4 * (NQ + 1)], F32,
                         kind="ExternalInput").ap()
    ident = nc.dram_tensor("ident", [128, 128], BF16, kind="ExternalInput").ap()
    msk = nc.dram_tensor("msk", [128, 4 * 512], BF16, kind="ExternalInput").ap()
    ones = nc.dram_tensor("ones", [128, 128], BF16, kind="ExternalInput").ap()
    outT = nc.dram_tensor("outT", [D, S], F32, kind="ExternalOutput").ap()

    with tile.TileContext(nc) as tc:
        from contextlib import ExitStack
        with ExitStack() as ctx:
            consts = ctx.enter_context(tc.tile_pool(name="consts", bufs=1))
            wpool = ctx.enter_context(tc.tile_pool(name="w", bufs=1))
            xpool = ctx.enter_context(tc.tile_pool(name="xT", bufs=2))
            qt_pool = ctx.enter_context(tc.tile_pool(name="qt", bufs=1))
            yt_pool = ctx.enter_context(tc.tile_pool(name="yt", bufs=1))
            v_pool = ctx.enter_context(tc.tile_pool(name="vrow", bufs=1))
            sq_pool = ctx.enter_context(tc.tile_pool(name="sq", bufs=3))
            st_pool = ctx.enter_context(tc.tile_pool(name="stat", bufs=3))
            tv_pool = ctx.enter_context(tc.tile_pool(name="tv", bufs=2))
            ro_pool = ctx.enter_context(tc.tile_pool(name="ro", bufs=2))
            pt_pool = ctx.enter_context(tc.tile_pool(name="ptile", bufs=6))
            rn_pool = ctx.enter_context(tc.tile_pool(name="rn", bufs=3))
            ob_pool = ctx.enter_context(tc.tile_pool(name="ob", bufs=4))
            # PSUM: pA 2x[128,1024] (4 banks) + pB 1x[128,1024] (2 banks)
            # + pC 2x[128,512] (2 banks) = all 8 banks.
            pA = ctx.enter_context(tc.tile_pool(name="pA", bufs=2, space="PSUM"))
            pB = ctx.enter_context(tc.tile_pool(name="pB", bufs=1, space="PSUM"))
            pC = ctx.enter_context(tc.tile_pool(name="pC", bufs=2, space="PSUM"))

            # ---- weight/const DMAs interleaved with the first xT block so
            # the first Q matmuls start ~2us in.
            xTr = xT.rearrange("(n p) m -> p n m", p=128)
            wqr = wq.rearrange("(n p) m -> p n m", p=128)
            wkvr = wkv.rearrange("(n p) m -> p n m", p=128)
            wq_sb = wpool.tile([128, NDC, DQ], BF16, tag="wq")
            wkv_sb = wpool.tile([128, NDC, 2 * HD], BF16, tag="wkv")

            x_blocks = [None] * NBLK
            x_blocks[0] = xpool.tile([128, NDC, 512], BF16, name="xb0",
                                     tag="xb")
            ident_sb = consts.tile([128, 128], BF16, tag="ident")
            nc.sync.dma_start(ident_sb[:], ident)
            nc.sync.dma_start(wq_sb[:, 0:2, :], wqr[:, 0:2, :])
            nc.sync.dma_start(x_blocks[0][:, 0:2, :], xTr[:, 0:2, 0:512])
            nc.sync.dma_start(wq_sb[:, 2:4, :], wqr[:, 2:4, :])
            nc.sync.dma_start(x_blocks[0][:, 2:4, :], xTr[:, 2:4, 0:512])
            nc.sync.dma_start(wq_sb[:, 4:8, :], wqr[:, 4:8, :])
            nc.sync.dma_start(x_blocks[0][:, 4:8, :], xTr[:, 4:8, 0:512])
            nc.sync.dma_start(wq_sb[:, 8:NDC, :], wqr[:, 8:NDC, :])
            nc.sync.dma_start(x_blocks[0][:, 8:NDC, :], xTr[:, 8:NDC, 0:512])
            nc.sync.dma_start(wkv_sb[:], wkvr)

            cst_sb = consts.tile([128, NST, 2 * HD], BF16, tag="cst")
            nc.sync.dma_start(cst_sb[:],
                              cst.rearrange("(n p) m -> p n m", p=128))
            qgb_sb = consts.tile([128, 4 * (NQ + 1)], F32, tag="qgb")
            nc.sync.dma_start(qgb_sb[:], qgb)
            ones_sb = consts.tile([128, 128], BF16, tag="ones")
            nc.sync.dma_start(ones_sb[:], ones)
            msk_sb = consts.tile([128, 4, 512], BF16, tag="msk")
            nc.sync.dma_start(msk_sb[:], msk.rearrange("p (m c) -> p m c", c=512))
            wo_sb = wpool.tile([128, NQ, D], BF16, tag="wo")
            nc.sync.dma_start(wo_sb[:], wo.rearrange("(n p) m -> p n m", p=128))

            # HAM warmup: ~4.5us of PE work with NO input dependency (reads
            # an uninitialized SBUF scratch tile) so the clock gate is at 8/8
            # and the PE pipeline primed when the first real matmuls arrive.
            wsrc = consts.tile([128, 128], BF16, tag="wsrc")
            nc.vector.memset(wsrc[:], 1.0)
            warm = pC.tile([128, 512], F32, name="warm", tag="c")
            for i in range(60):
                nc.tensor.matmul(warm[:, 0:128], wsrc[:], wsrc[:],
                                 start=True, stop=True)

            qt_all = qt_pool.tile([128, NH1, S], BF16, name="qt_all",
                                  tag="qt_all")
            yt_tiles = [yt_pool.tile([128, S], BF16, name=f"yt{h}", tag=f"yt{h}")
                        for h in range(NQ)]
            v_tiles = [v_pool.tile([128, 132], BF16, name=f"v{st}", tag=f"v{st}")
                       for st in range(NST)]

            # ---- Phase 1: projections + rms-norm + rope + transpose ----
            # The PE transposes of s-tile st are deferred until the next
            # s-tile's projection matmuls have been emitted, so the DVE rope
            # chain has a full tile of slack before the PE needs its output.
            tr_state = {"pend": []}

            def flush_one_tr():
                st, ro5 = tr_state["pend"].pop(0)
                bt = pB.tile([128, 1024], F32, name="bt", tag="b")
                ptv = bt[:, 0:NH1 * 64].bitcast(BF16)  # [128, 640] bf16
                for i in range(NH1):
                    nc.tensor.transpose(ptv[:, i * 128:(i + 1) * 128],
                                        ro5[:, i * HD:(i + 1) * HD],
                                        ident_sb[:])
                nc.scalar.copy(
                    qt_all[:, :, st * 128:(st + 1) * 128],
                    ptv.rearrange("p (h c) -> p h c", c=128))

            def flush_tr():
                while tr_state["pend"]:
                    flush_one_tr()

            def process_stile(st, xb, st4):
                at = pA.tile([128, 1024], F32, name="at", tag="a")
                pq = at[:, 0:DQ]
                pkv = at[:, DQ:DQ + 2 * HD]
                for dc in range(NDC):
                    nc.tensor.matmul(pq, xb[:, dc, st4 * 128:(st4 + 1) * 128],
                                     wq_sb[:, dc, :],
                                     start=dc == 0, stop=dc == NDC - 1)
                for dc in range(NDC):
                    nc.tensor.matmul(pkv, xb[:, dc, st4 * 128:(st4 + 1) * 128],
                                     wkv_sb[:, dc, :],
                                     start=dc == 0, stop=dc == NDC - 1)
                if len(tr_state["pend"]) >= 2:
                    flush_one_tr()

                # evacuate PSUM on ScalarE (frees the accumulator banks
                # fast; the slow DVE chain then runs from SBUF)
                qkv = sq_pool.tile([128, DQ + 2 * HD], BF16, tag="qkv")
                with tc.high_priority():
                    nc.scalar.copy(qkv[:, 0:DQ], pq)
                    nc.vector.tensor_copy(qkv[:, DQ:DQ + 2 * HD], pkv)

                # V row tile [v | 1] for the AV' matmuls
                nc.vector.tensor_copy(v_tiles[st][:, 0:HD],
                                      qkv[:, DQ + HD:DQ + 2 * HD])
                nc.vector.tensor_copy(v_tiles[st][:, HD:HD + 1],
                                      ones_sb[:, 0:1])

                # RMS stats: one batched square + one grouped reduce
                sq = sq_pool.tile([128, NH1 * HD], F32, tag="sq")
                nc.scalar.activation(sq[:], qkv[:, 0:DQ + HD], AF.Square)
                ssq = st_pool.tile([128, NH1], F32, tag="ssq")
                sq_g = AP(sq.tensor, sq.offset, [sq.ap[0], [HD, NH1], [1, HD]])
                nc.vector.tensor_reduce(ssq[:], sq_g, axis=AXL.X, op=ALU.add)

                # rinv = (mean(q^2)+eps)**-0.5 on DVE (bit-trick + 2 Newton)
                m = st_pool.tile([128, NH1], F32, tag="m")
                nc.vector.tensor_scalar(m[:], ssq[:], 1.0 / HD, RMS_EPS,
                                        op0=ALU.mult, op1=ALU.add)
                y0 = st_pool.tile([128, NH1], F32, tag="y0")
                nc.vector.tensor_scalar(y0[:].bitcast(I32),
                                        m[:].bitcast(I32), 1, None,
                                        op0=ALU.arith_shift_right)
                nc.vector.tensor_scalar(y0[:].bitcast(I32),
                                        y0[:].bitcast(I32),
                                        -1, 0x5F3759DF,
                                        op0=ALU.mult, op1=ALU.add)
                rinv = y0
                for _ in range(1):
                    aa = st_pool.tile([128, NH1], F32, tag="nr_a")
                    nc.vector.tensor_mul(aa[:], rinv[:], rinv[:])
                    nc.vector.tensor_mul(aa[:], aa[:], m[:])
                    nc.vector.tensor_scalar(aa[:], aa[:], -0.5, 1.5,
                                            op0=ALU.mult, op1=ALU.add)
                    nxt = st_pool.tile([128, NH1], F32, tag="nr_y")
                    nc.vector.tensor_mul(nxt[:], rinv[:], aa[:])
                    rinv = nxt
                nc.vector.tensor_mul(rinv[:], rinv[:], qgb_sb[:, 0:NH1])

                # rope straight from PSUM: per head [t|v] = (q*rinv)*[c|c|-s|s]
                cst_t = cst_sb[:, st, :]
                tv5 = tv_pool.tile([128, NH1 * 2 * HD], BF16, tag="tv5")
                for i in range(NH1):
                    q_ap = qkv[:, i * HD:(i + 1) * HD]
                    q_rep = AP(q_ap.tensor, q_ap.offset,
                               [q_ap.ap[0], [0, 2], [1, HD]])
                    nc.vector.scalar_tensor_tensor(
                        tv5[:, i * 2 * HD:(i + 1) * 2 * HD],
                        q_rep, rinv[:, i:i + 1], cst_t,
                        op0=ALU.mult, op1=ALU.mult)
                ro5 = ro_pool.tile([128, NH1 * HD], BF16, tag="ro5")
                b5 = tv5[:]
                t_view = AP(b5.tensor, b5.offset,
                            [b5.ap[0], [2 * HD, NH1], [H, 2], [1, H]])
                v_view = AP(b5.tensor, b5.offset + HD + H,
                            [b5.ap[0], [2 * HD, NH1], [-H, 2], [1, H]])
                r5 = ro5[:]
                o_view = AP(r5.tensor, r5.offset,
                            [r5.ap[0], [HD, NH1], [H, 2], [1, H]])
                nc.vector.tensor_add(o_view, t_view, v_view)
                tr_state["pend"].append((st, ro5))

            # ---- Phases interleaved: projections block b -> attention
            # qc=b -> outproj qc=b-1 as PE filler between heads ----
            kt_row = qt_all[:, NQ, :]

            # out-projection dribbled one 128x512 tile at a time between
            # attention score groups (PE filler while ScalarE exps run);
            # outT DMA batched per 4 tiles.
            op_state = {"pending": [], "ob": None, "row": 0}

            def push_outproj(qcp):
                op_state["pending"].extend((qcp, dt) for dt in range(NST))

            def emit_outproj_unit(copy_eng="vector"):
                if not op_state["pending"]:
                    return
                qcp, dt = op_state["pending"].pop(0)
                if op_state["row"] == 0:
                    op_state["ob"] = ob_pool.tile([128, 4, 512], F32,
                                                  name="ob", tag="ob")
                po = pC.tile([128, 512], F32, name="po", tag="c")
                for dqc in range(NQ):
                    nc.tensor.matmul(
                        po[:], wo_sb[:, dqc, dt * 128:(dt + 1) * 128],
                        yt_tiles[dqc][:, qcp * 512:(qcp + 1) * 512],
                        start=(dqc == 0), stop=(dqc == NQ - 1))
                if copy_eng == "scalar":
                    nc.scalar.copy(op_state["ob"][:, op_state["row"], :], po[:])
                else:
                    nc.vector.tensor_copy(
                        op_state["ob"][:, op_state["row"], :], po[:])
                op_state["row"] += 1
                if op_state["row"] == 4:
                    op_state["row"] = 0
                    nc.sync.dma_start(
                        outT[(dt - 3) * 128:(dt + 1) * 128,
                             qcp * 512:(qcp + 1) * 512].rearrange(
                                 "(n p) m -> p n m", p=128),
                        op_state["ob"][:])

            work_q = []  # pending s-tile closures (consumed as PE filler)

            def fill_slot():
                if work_q:
                    work_q.pop(0)()
                else:
                    emit_outproj_unit()

            def attention_head(qc, h):
                n_kt = 4 * qc + 4
                n_groups = n_kt // 2
                qs = qt_all[:, h, qc * 512:(qc + 1) * 512]

                def off_of(kt):
                    return max(0, kt - 4 * qc) * 128

                def emit_scores_group(g):
                    sp = pA.tile([128, 1024], F32, name="sp", tag="a")
                    ptp = pt_pool.tile([128, 2, 512], BF16, name="ptp",
                                       tag="ptp")
                    for u in (0, 1):
                        kt = 2 * g + u
                        off = off_of(kt)
                        nc.tensor.matmul(
                            sp[:, u * 512 + off:(u + 1) * 512],
                            kt_row[:, kt * 128:(kt + 1) * 128],
                            qs[:, off:512], start=True, stop=True)
                    if 2 * g + 1 < 4 * qc:  # both tiles non-diagonal
                        nc.scalar.activation(
                            ptp[:].rearrange("p a b -> p (a b)"),
                            sp[:], AF.Exp, scale=scale)
                    else:
                        for u in (0, 1):
                            kt = 2 * g + u
                            off = off_of(kt)
                            nc.scalar.activation(ptp[:, u, off:512],
                                                 sp[:, u * 512 + off:(u + 1) * 512],
                                                 AF.Exp, scale=scale)
                    for u in (0, 1):
                        kt = 2 * g + u
                        mdiag = kt - 4 * qc
                        if mdiag >= 0:
                            off = mdiag * 128
                            nc.vector.tensor_mul(
                                ptp[:, u, off:512], ptp[:, u, off:512],
                                msk_sb[:, mdiag, off:512])
                    return ptp

                def emit_av_group(g, ptp, pyp):
                    for u in (0, 1):
                        kt = 2 * g + u
                        j0 = max(0, kt - 4 * qc)
                        for j in range(j0, 4):
                            nc.tensor.matmul(
                                pyp[:, PYOFF[j]:PYOFF[j] + HD + 1],
                                ptp[:, u, j * 128:(j + 1) * 128],
                                v_tiles[kt][:, 0:HD + 1],
                                start=(kt == 0 and j in (0, 3)),
                                stop=(kt == 4 * qc + j),
                                skip_group_check=True)

                prev = emit_scores_group(0)
                fill_slot()
                # start=True on any matmul clears has_written for its whole
                # PSUM bank, which would wipe sibling accumulators sharing
                # the bank -- so zero the regions once and accumulate with
                # start=False throughout.
                pyp = pB.tile([128, 1024], F32, name="pyp", tag="b")
                for g in range(1, n_groups):
                    cur = emit_scores_group(g)
                    emit_av_group(g - 1, prev, pyp)
                    prev = cur
                    if qc < 2 or g % 2 == 1:
                        fill_slot()
                emit_av_group(n_groups - 1, prev, pyp)

                # normalize: rcp of the 4 denominator columns, then per-
                # partition scale of each [q,hd] block; transpose to [hd,q].
                rcp = rn_pool.tile([128, 4], F32, tag="rcp")
                r3 = AP(pyp.tensor, pyp.offset + HD,
                        [pyp.ap[0], [PYOFF[1] - PYOFF[0], 3]])
                nc.vector.reciprocal(out=rcp[:, 0:3], in_=r3)
                nc.vector.reciprocal(out=rcp[:, 3:4],
                                     in_=pyp[:, PYOFF[3] + HD:PYOFF[3] + HD + 1])
                yn = rn_pool.tile([128, NQ * HD], BF16, tag="yn")
                for j in range(4):
                    nc.vector.tensor_scalar(
                        yn[:, j * HD:(j + 1) * HD],
                        pyp[:, PYOFF[j]:PYOFF[j] + HD],
                        rcp[:, j:j + 1], None, op0=ALU.mult)
                ytr = pyp[:, 656:912].bitcast(BF16)  # [128, 512] bf16
                for j in range(4):
                    nc.tensor.transpose(ytr[:, j * 128:(j + 1) * 128],
                                        yn[:, j * HD:(j + 1) * HD],
                                        ident_sb[:])
                nc.vector.tensor_copy(
                    yt_tiles[h][:, qc * 512:(qc + 1) * 512], ytr)

            def prefetch_xb(b):
                x_blocks[b] = xpool.tile([128, NDC, 512], BF16,
                                         name=f"xb{b}", tag="xb")
                nc.sync.dma_start(x_blocks[b][:, 0:8, :],
                                  xTr[:, 0:8, b * 512:(b + 1) * 512])
                nc.sync.dma_start(x_blocks[b][:, 8:NDC, :],
                                  xTr[:, 8:NDC, b * 512:(b + 1) * 512])

            # stage A: s-tiles 0-4 directly (one past block 0, so the
            # 1-deep transpose deferral never leaves a tile the next stage's
            # heads need pending at a stage boundary)
            prefetch_xb(1)
            for st in range(5):
                process_stile(st, x_blocks[st // 4], st % 4)

            # stages B..E: attention for qc in order; projection s-tiles of
            # later blocks and out-projection tiles serve as PE filler inside
            # the heads (supply matched to the heads' fill slots).
            STAGE_STILES = {0: range(5, 12), 1: range(12, 16),
                            2: range(0, 0), 3: range(0, 0)}
            for qc in range(NQC):
                if qc == 0:
                    prefetch_xb(2)
                    prefetch_xb(3)
                for st in STAGE_STILES[qc]:
                    work_q.append(
                        lambda st=st: process_stile(st, x_blocks[st // 4],
                                                    st % 4))
                if qc == NQC - 1:
                    flush_tr()  # qc3's heads need st15's qt
                for h in range(NQ):
                    attention_head(qc, h)
                while work_q:  # any unconsumed s-tiles must land before the
                    work_q.pop(0)()  # next stage's heads read their qt/v
                push_outproj(qc)
            while op_state["pending"]:
                emit_outproj_unit()

    nc.compile()
    return nc


def get_nc():
    if "nc" not in _NC_CACHE:
        _NC_CACHE["nc"] = build_kernel()
    return _NC_CACHE["nc"]


def rope_tables(S=S, HD=HD):
    """Packed rope table [S, 2*HD]: [c | c | -s | s]."""
    f = 1.0 / (10000.0 ** (np.arange(0, HD, 2, dtype=np.float32) / HD))
    fr = np.outer(np.arange(S, dtype=np.float32), f)
    c = np.cos(fr).astype(np.float32)
    s = np.sin(fr).astype(np.float32)
    return np.concatenate([c, c, -s, s], axis=1)


def make_in_maps(x, Wq, Wk, Wv, Wo, qg):
    x = np.asarray(x, np.float32)
    Wq = np.asarray(Wq, np.float32)
    Wk = np.asarray(Wk, np.float32)
    Wv = np.asarray(Wv, np.float32)
    Wo = np.asarray(Wo, np.float32)
    qg = np.asarray(qg, np.float32)
    cst = rope_tables()
    ident = np.eye(128, dtype=BF16NP)
    ones = np.ones((128, 128), dtype=BF16NP)
    pp, ff = np.arange(128)[:, None], np.arange(512)[None, :]
    msk = np.concatenate(
        [(ff >= pp + 128 * mm).astype(BF16NP) for mm in range(4)], axis=1)
    xT = [np.ascontiguousarray(x[b].T).astype(BF16NP) for b in range(NB)]
    in_maps = []
    for b in range(NB):
        for tp in range(NTP):
            qgb_row = np.broadcast_to(
                np.concatenate([qg[tp * NQ:(tp + 1) * NQ],
                                [np.float32(1.0)]] * 4)[None, :].astype(np.float32),
                (128, 4 * (NQ + 1))).copy()
            wkv = np.concatenate([
                Wk[tp * HD:(tp + 1) * HD, :].T,
                Wv[tp * HD:(tp + 1) * HD, :].T], axis=1)
            in_maps.append({
                "xT": xT[b],
                "wq": np.ascontiguousarray(
                    Wq[tp * DQ:(tp + 1) * DQ, :].T).astype(BF16NP),
                "wkv": np.ascontiguousarray(wkv).astype(BF16NP),
                "wo": np.ascontiguousarray(
                    Wo[:, tp * DQ:(tp + 1) * DQ].T).astype(BF16NP),
                "cst": cst.astype(BF16NP),
                "qgb": qgb_row,
                "ident": ident,
                "ones": ones,
                "msk": msk,
            })
    return in_maps


def run(x, Wq, Wk, Wv, Wo, qg, trace=False, **trace_kwargs):
    nc = get_nc()
    in_maps = make_in_maps(x, Wq, Wk, Wv, Wo, qg)
    res = run_bass_kernel_spmd(nc, in_maps, core_ids=list(range(NCORES)),
                               trace=trace, **trace_kwargs)
    out = np.empty((NB, S, D), np.float32)
    for b in range(NB):
        acc = res.results[b * NTP]["outT"].astype(np.float32)
        for tp in range(1, NTP):
            acc = acc + res.results[b * NTP + tp]["outT"]
        out[b] = acc.T
    return out, res


def kernel(x, Wq, Wk, Wv, Wo, qg):
    out, _ = run(x, Wq, Wk, Wv, Wo, qg)
    return out
